# revision 1
# baseline (speedup 1.0000x reference)
"""Trainium2 Bass kernel: per-gaussian 3x3 covariance from quaternion+scale.

out_n = R_n diag((|s_n|+eps)^2) R_n^T  with R_n from normalized quaternion.

Math: with raw (unnormalized) quaternion q=(w,x,y,z), n2=|q|^2, the matrix
M = n2*R has polynomial entries (no normalization needed):
  M00 = n2-(2y^2+2z^2)   M01 = 2xy-2wz   M02 = 2xz+2wy
  M10 = 2xy+2wz          M11 = n2-(2x^2+2z^2)   M12 = 2yz-2wx
  M20 = 2xz-2wy          M21 = 2yz+2wx   M22 = n2-(2x^2+2y^2)
With u_j = s_j/n2, B = M diag(u) = R diag(s_j), so out = B B^T
(column signs cancel in B B^T, and eps=1e-8 is numerically negligible).

Layout: host transposes inputs to component-planar planes per core, device
computes 6 unique output planes (symmetric), host reassembles [N,3,3].
"""

import numpy as np

N_TOTAL = 4_000_000
N_CORES = 8
NC_RAW = N_TOTAL // N_CORES  # 500_000
P = 128
F = -(-NC_RAW // P)          # 3907 elements per partition
NC_PAD = P * F               # 500_096
W = 512                      # tile width along free dim

_COMPILED = None


def _build():
    import concourse.bacc as bacc
    import concourse.mybir as mybir
    from concourse import tile

    fp32 = mybir.dt.float32
    Alu = mybir.AluOpType
    Act = mybir.ActivationFunctionType
    SQ2 = float(np.sqrt(2.0))

    nc = bacc.Bacc("TRN2", target_bir_lowering=False, debug=False,
                   num_devices=N_CORES)
    qt = nc.dram_tensor("qt", [4, NC_PAD], fp32, kind="ExternalInput")
    st = nc.dram_tensor("st", [3, NC_PAD], fp32, kind="ExternalInput")
    ot = nc.dram_tensor("ot", [6, NC_PAD], fp32, kind="ExternalOutput")

    qv = qt.ap().rearrange("c (p f) -> c p f", p=P)
    sv = st.ap().rearrange("c (p f) -> c p f", p=P)
    ov = ot.ap().rearrange("c (p f) -> c p f", p=P)

    with tile.TileContext(nc) as tc:
        with tc.tile_pool(name="pool", bufs=1) as pool:
            V = nc.vector
            A = nc.scalar

            def new(tag, w, bufs=1):
                return pool.tile([P, w], fp32, tag=tag, name=tag, bufs=bufs)

            off = 0
            while off < F:
                w = min(W, F - off)
                sl = slice(off, off + w)

                # ---- loads (component planes) ----
                tq = [new(f"q{i}", w, bufs=2) for i in range(4)]  # w,x,y,z
                for i in range(4):
                    nc.sync.dma_start(out=tq[i][:], in_=qv[i, :, sl])
                ts = [new(f"s{i}", w, bufs=2) for i in range(3)]
                for i in range(3):
                    nc.sync.dma_start(out=ts[i][:], in_=sv[i, :, sl])
                tw, tx, ty, tz = tq

                # ---- squares on ACT: sw=w^2, sx2=2x^2, sy2=2y^2, sz2=2z^2
                sw = new("sw", w)
                A.activation(sw[:], tw[:], Act.Square)
                sx2 = new("sx2", w)
                A.activation(sx2[:], tx[:], Act.Square, scale=SQ2)
                sy2 = new("sy2", w)
                A.activation(sy2[:], ty[:], Act.Square, scale=SQ2)
                sz2 = new("sz2", w)
                A.activation(sz2[:], tz[:], Act.Square, scale=SQ2)

                # ---- doubled cross products via fused (a*2)*b
                xy2 = new("xy2", w)
                V.scalar_tensor_tensor(xy2[:], tx[:], 2.0, ty[:], Alu.mult, Alu.mult)
                xz2 = new("xz2", w)
                V.scalar_tensor_tensor(xz2[:], tx[:], 2.0, tz[:], Alu.mult, Alu.mult)
                yz2 = new("yz2", w)
                V.scalar_tensor_tensor(yz2[:], ty[:], 2.0, tz[:], Alu.mult, Alu.mult)
                wx2 = new("wx2", w)
                V.scalar_tensor_tensor(wx2[:], tw[:], 2.0, tx[:], Alu.mult, Alu.mult)
                wy2 = new("wy2", w)
                V.scalar_tensor_tensor(wy2[:], tw[:], 2.0, ty[:], Alu.mult, Alu.mult)
                wz2 = new("wz2", w)
                V.scalar_tensor_tensor(wz2[:], tw[:], 2.0, tz[:], Alu.mult, Alu.mult)

                # ---- diagonal helpers
                e2 = new("e2", w)
                V.tensor_tensor(e2[:], sx2[:], sy2[:], Alu.add)      # 2x²+2y²
                t1 = new("t1", w)
                V.tensor_tensor(t1[:], e2[:], sz2[:], Alu.add)       # 2(x²+y²+z²)
                e0 = new("e0", w)
                V.tensor_tensor(e0[:], t1[:], sx2[:], Alu.subtract)  # 2y²+2z²
                e1 = new("e1", w)
                V.tensor_tensor(e1[:], t1[:], sy2[:], Alu.subtract)  # 2x²+2z²
                n2 = new("n2", w)
                V.scalar_tensor_tensor(n2[:], t1[:], 0.5, sw[:], Alu.mult, Alu.add)

                inv = new("inv", w)
                V.reciprocal_approx_fast(out=inv[:], in_=n2[:])

                u = [new(f"u{j}", w) for j in range(3)]
                for j in range(3):
                    V.tensor_tensor(u[j][:], ts[j][:], inv[:], Alu.mult)

                # ---- M entries
                m = {}
                for (key, aa, bb, op) in (
                    ("00", n2, e0, Alu.subtract),
                    ("11", n2, e1, Alu.subtract),
                    ("22", n2, e2, Alu.subtract),
                    ("01", xy2, wz2, Alu.subtract),
                    ("10", xy2, wz2, Alu.add),
                    ("02", xz2, wy2, Alu.add),
                    ("20", xz2, wy2, Alu.subtract),
                    ("12", yz2, wx2, Alu.subtract),
                    ("21", yz2, wx2, Alu.add),
                ):
                    t = new(f"m{key}", w)
                    V.tensor_tensor(t[:], aa[:], bb[:], op)
                    m[key] = t

                # ---- B = M diag(u)  (B == R diag(s))
                b = {}
                for i in range(3):
                    for j in range(3):
                        t = new(f"b{i}{j}", w)
                        V.tensor_tensor(t[:], m[f"{i}{j}"][:], u[j][:], Alu.mult)
                        b[(i, j)] = t

                # ---- diagonal outputs via ACT squares
                couts = {}
                for i in range(3):
                    d0 = new(f"d{i}0", w)
                    A.activation(d0[:], b[(i, 0)][:], Act.Square)
                    d1 = new(f"d{i}1", w)
                    A.activation(d1[:], b[(i, 1)][:], Act.Square)
                    d2 = new(f"d{i}2", w)
                    A.activation(d2[:], b[(i, 2)][:], Act.Square)
                    ca = new(f"ca{i}", w)
                    V.tensor_tensor(ca[:], d0[:], d1[:], Alu.add)
                    cd = new(f"c{i}{i}", w, bufs=2)
                    V.tensor_tensor(cd[:], ca[:], d2[:], Alu.add)
                    couts[(i, i)] = cd

                # ---- off-diagonal outputs
                for (i, k) in ((0, 1), (0, 2), (1, 2)):
                    p0 = new(f"p{i}{k}", w)
                    V.tensor_tensor(p0[:], b[(i, 0)][:], b[(k, 0)][:], Alu.mult)
                    p1 = new(f"q{i}{k}", w)
                    V.tensor_tensor(p1[:], b[(i, 1)][:], b[(k, 1)][:], Alu.mult)
                    p01 = new(f"pq{i}{k}", w)
                    V.tensor_tensor(p01[:], p0[:], p1[:], Alu.add)
                    p2 = new(f"r{i}{k}", w)
                    V.tensor_tensor(p2[:], b[(i, 2)][:], b[(k, 2)][:], Alu.mult)
                    co = new(f"c{i}{k}", w, bufs=2)
                    V.tensor_tensor(co[:], p01[:], p2[:], Alu.add)
                    couts[(i, k)] = co

                # ---- stores: plane order c00,c01,c02,c11,c12,c22
                for plane, key in enumerate(((0, 0), (0, 1), (0, 2),
                                             (1, 1), (1, 2), (2, 2))):
                    nc.sync.dma_start(out=ov[plane, :, sl], in_=couts[key][:])

                off += w

    nc.compile()
    return nc


def _get_compiled():
    global _COMPILED
    if _COMPILED is None:
        _COMPILED = _build()
    return _COMPILED


def kernel(quaternion, scale):
    from concourse.bass_utils import run_bass_kernel_spmd

    q = np.ascontiguousarray(np.asarray(quaternion, dtype=np.float32))
    s = np.ascontiguousarray(np.asarray(scale, dtype=np.float32))
    assert q.shape == (N_TOTAL, 4) and s.shape == (N_TOTAL, 3)

    in_maps = []
    for c in range(N_CORES):
        lo, hi = c * NC_RAW, (c + 1) * NC_RAW
        qt = np.empty((4, NC_PAD), np.float32)
        qt[:, :NC_RAW] = q[lo:hi].T
        qt[0, NC_RAW:] = 1.0  # pad with identity quaternion
        qt[1:, NC_RAW:] = 0.0
        stt = np.ones((3, NC_PAD), np.float32)
        stt[:, :NC_RAW] = s[lo:hi].T
        in_maps.append({"qt": qt, "st": stt})

    nc = _get_compiled()
    res = run_bass_kernel_spmd(nc, in_maps, core_ids=list(range(N_CORES)))

    out = np.empty((N_TOTAL, 3, 3), np.float32)
    # plane -> (i,j) positions (symmetric duplicates share a plane)
    placement = ((0, (0, 0)), (1, (0, 1)), (2, (0, 2)),
                 (1, (1, 0)), (3, (1, 1)), (4, (1, 2)),
                 (2, (2, 0)), (4, (2, 1)), (5, (2, 2)))
    for c in range(N_CORES):
        o6 = res.results[c]["ot"]
        lo = c * NC_RAW
        for plane, (i, j) in placement:
            out[lo:lo + NC_RAW, i, j] = o6[plane, :NC_RAW]
    return out


# revision 3
# speedup vs baseline: 1.7294x; 1.7294x over previous
"""Trainium2 Bass kernel: per-gaussian 3x3 covariance from quaternion+scale.

out_n = R_n diag((|s_n|+eps)^2) R_n^T  with R_n from normalized quaternion.

Math: with raw (unnormalized) quaternion q=(w,x,y,z), n2=|q|^2, the matrix
M = n2*R has polynomial entries (no normalization needed):
  M00 = n2-(2y^2+2z^2)   M01 = 2xy-2wz   M02 = 2xz+2wy
  M10 = 2xy+2wz          M11 = n2-(2x^2+2z^2)   M12 = 2yz-2wx
  M20 = 2xz-2wy          M21 = 2yz+2wx   M22 = n2-(2x^2+2y^2)
With u_j = s_j/n2, B = M diag(u) = R diag(s_j), so out = B B^T
(column signs cancel in B B^T, and eps=1e-8 is numerically negligible).

Layout: host transposes inputs to component-planar planes per core, device
computes 6 unique output planes (symmetric), host reassembles [N,3,3].
"""

import numpy as np

N_TOTAL = 4_000_000
N_CORES = 8
NC_RAW = N_TOTAL // N_CORES  # 500_000
P = 128
F = -(-NC_RAW // P)          # 3907 elements per partition
NC_PAD = P * F               # 500_096
W = 512                      # tile width along free dim

_COMPILED = None


def _build(repeat=1):
    import contextlib
    import concourse.bacc as bacc
    import concourse.mybir as mybir
    from concourse import tile

    fp32 = mybir.dt.float32
    Alu = mybir.AluOpType
    Act = mybir.ActivationFunctionType
    SQ2 = float(np.sqrt(2.0))

    nc = bacc.Bacc("TRN2", target_bir_lowering=False, debug=False,
                   num_devices=N_CORES)
    qt = nc.dram_tensor("qt", [4, NC_PAD], fp32, kind="ExternalInput")
    st = nc.dram_tensor("st", [3, NC_PAD], fp32, kind="ExternalInput")
    ot = nc.dram_tensor("ot", [6, NC_PAD], fp32, kind="ExternalOutput")

    qv = qt.ap().rearrange("c (p f) -> c p f", p=P)
    sv = st.ap().rearrange("c (p f) -> c p f", p=P)
    ov = ot.ap().rearrange("c (p f) -> c p f", p=P)

    with tile.TileContext(nc) as tc:
        loop_ctx = tc.For_i(0, repeat, 1) if repeat > 1 else contextlib.nullcontext()
        with loop_ctx, tc.tile_pool(name="pool", bufs=1) as pool:
            V = nc.vector
            A = nc.scalar

            def new(tag, w, bufs=1):
                return pool.tile([P, w], fp32, tag=tag, name=tag, bufs=bufs)

            off = 0
            while off < F:
                w = min(W, F - off)
                sl = slice(off, off + w)

                # ---- loads (component planes) ----
                tq = [new(f"q{i}", w, bufs=2) for i in range(4)]  # w,x,y,z
                for i in range(4):
                    nc.sync.dma_start(out=tq[i][:], in_=qv[i, :, sl])
                ts = [new(f"s{i}", w, bufs=2) for i in range(3)]
                for i in range(3):
                    nc.sync.dma_start(out=ts[i][:], in_=sv[i, :, sl])
                tw, tx, ty, tz = tq

                # ---- squares on ACT: sw=w^2, sx2=2x^2, sy2=2y^2, sz2=2z^2
                sw = new("sw", w)
                A.activation(sw[:], tw[:], Act.Square)
                sx2 = new("sx2", w)
                A.activation(sx2[:], tx[:], Act.Square, scale=SQ2)
                sy2 = new("sy2", w)
                A.activation(sy2[:], ty[:], Act.Square, scale=SQ2)
                sz2 = new("sz2", w)
                A.activation(sz2[:], tz[:], Act.Square, scale=SQ2)

                # ---- doubled cross products via fused (a*2)*b
                xy2 = new("xy2", w)
                V.scalar_tensor_tensor(xy2[:], tx[:], 2.0, ty[:], Alu.mult, Alu.mult)
                xz2 = new("xz2", w)
                V.scalar_tensor_tensor(xz2[:], tx[:], 2.0, tz[:], Alu.mult, Alu.mult)
                yz2 = new("yz2", w)
                V.scalar_tensor_tensor(yz2[:], ty[:], 2.0, tz[:], Alu.mult, Alu.mult)
                wx2 = new("wx2", w)
                V.scalar_tensor_tensor(wx2[:], tw[:], 2.0, tx[:], Alu.mult, Alu.mult)
                wy2 = new("wy2", w)
                V.scalar_tensor_tensor(wy2[:], tw[:], 2.0, ty[:], Alu.mult, Alu.mult)
                wz2 = new("wz2", w)
                V.scalar_tensor_tensor(wz2[:], tw[:], 2.0, tz[:], Alu.mult, Alu.mult)

                # ---- diagonal helpers
                e2 = new("e2", w)
                V.tensor_tensor(e2[:], sx2[:], sy2[:], Alu.add)      # 2x²+2y²
                t1 = new("t1", w)
                V.tensor_tensor(t1[:], e2[:], sz2[:], Alu.add)       # 2(x²+y²+z²)
                e0 = new("e0", w)
                V.tensor_tensor(e0[:], t1[:], sx2[:], Alu.subtract)  # 2y²+2z²
                e1 = new("e1", w)
                V.tensor_tensor(e1[:], t1[:], sy2[:], Alu.subtract)  # 2x²+2z²
                n2 = new("n2", w)
                V.scalar_tensor_tensor(n2[:], t1[:], 0.5, sw[:], Alu.mult, Alu.add)

                inv = new("inv", w)
                V.reciprocal_approx_fast(out=inv[:], in_=n2[:])

                u = [new(f"u{j}", w) for j in range(3)]
                for j in range(3):
                    V.tensor_tensor(u[j][:], ts[j][:], inv[:], Alu.mult)

                # ---- M entries
                m = {}
                for (key, aa, bb, op) in (
                    ("00", n2, e0, Alu.subtract),
                    ("11", n2, e1, Alu.subtract),
                    ("22", n2, e2, Alu.subtract),
                    ("01", xy2, wz2, Alu.subtract),
                    ("10", xy2, wz2, Alu.add),
                    ("02", xz2, wy2, Alu.add),
                    ("20", xz2, wy2, Alu.subtract),
                    ("12", yz2, wx2, Alu.subtract),
                    ("21", yz2, wx2, Alu.add),
                ):
                    t = new(f"m{key}", w)
                    V.tensor_tensor(t[:], aa[:], bb[:], op)
                    m[key] = t

                # ---- B = M diag(u)  (B == R diag(s))
                b = {}
                for i in range(3):
                    for j in range(3):
                        t = new(f"b{i}{j}", w)
                        V.tensor_tensor(t[:], m[f"{i}{j}"][:], u[j][:], Alu.mult)
                        b[(i, j)] = t

                # ---- diagonal outputs via ACT squares
                couts = {}
                for i in range(3):
                    d0 = new(f"d{i}0", w)
                    A.activation(d0[:], b[(i, 0)][:], Act.Square)
                    d1 = new(f"d{i}1", w)
                    A.activation(d1[:], b[(i, 1)][:], Act.Square)
                    d2 = new(f"d{i}2", w)
                    A.activation(d2[:], b[(i, 2)][:], Act.Square)
                    ca = new(f"ca{i}", w)
                    V.tensor_tensor(ca[:], d0[:], d1[:], Alu.add)
                    cd = new(f"c{i}{i}", w, bufs=2)
                    V.tensor_tensor(cd[:], ca[:], d2[:], Alu.add)
                    couts[(i, i)] = cd

                # ---- off-diagonal outputs
                for (i, k) in ((0, 1), (0, 2), (1, 2)):
                    p0 = new(f"p{i}{k}", w)
                    V.tensor_tensor(p0[:], b[(i, 0)][:], b[(k, 0)][:], Alu.mult)
                    p1 = new(f"q{i}{k}", w)
                    V.tensor_tensor(p1[:], b[(i, 1)][:], b[(k, 1)][:], Alu.mult)
                    p01 = new(f"pq{i}{k}", w)
                    V.tensor_tensor(p01[:], p0[:], p1[:], Alu.add)
                    p2 = new(f"r{i}{k}", w)
                    V.tensor_tensor(p2[:], b[(i, 2)][:], b[(k, 2)][:], Alu.mult)
                    co = new(f"c{i}{k}", w, bufs=2)
                    V.tensor_tensor(co[:], p01[:], p2[:], Alu.add)
                    couts[(i, k)] = co

                # ---- stores: plane order c00,c01,c02,c11,c12,c22
                for plane, key in enumerate(((0, 0), (0, 1), (0, 2),
                                             (1, 1), (1, 2), (2, 2))):
                    nc.sync.dma_start(out=ov[plane, :, sl], in_=couts[key][:])

                off += w

    nc.compile()
    return nc


def _get_compiled():
    global _COMPILED
    if _COMPILED is None:
        _COMPILED = _build()
    return _COMPILED


def kernel(quaternion, scale):
    from concourse.bass_utils import run_bass_kernel_spmd

    q = np.ascontiguousarray(np.asarray(quaternion, dtype=np.float32))
    s = np.ascontiguousarray(np.asarray(scale, dtype=np.float32))
    assert q.shape == (N_TOTAL, 4) and s.shape == (N_TOTAL, 3)

    in_maps = []
    for c in range(N_CORES):
        lo, hi = c * NC_RAW, (c + 1) * NC_RAW
        qt = np.empty((4, NC_PAD), np.float32)
        qt[:, :NC_RAW] = q[lo:hi].T
        qt[0, NC_RAW:] = 1.0  # pad with identity quaternion
        qt[1:, NC_RAW:] = 0.0
        stt = np.ones((3, NC_PAD), np.float32)
        stt[:, :NC_RAW] = s[lo:hi].T
        in_maps.append({"qt": qt, "st": stt})

    nc = _get_compiled()
    res = run_bass_kernel_spmd(nc, in_maps, core_ids=list(range(N_CORES)))

    out = np.empty((N_TOTAL, 3, 3), np.float32)
    # plane -> (i,j) positions (symmetric duplicates share a plane)
    placement = ((0, (0, 0)), (1, (0, 1)), (2, (0, 2)),
                 (1, (1, 0)), (3, (1, 1)), (4, (1, 2)),
                 (2, (2, 0)), (4, (2, 1)), (5, (2, 2)))
    for c in range(N_CORES):
        o6 = res.results[c]["ot"]
        lo = c * NC_RAW
        for plane, (i, j) in placement:
            out[lo:lo + NC_RAW, i, j] = o6[plane, :NC_RAW]
    return out


# revision 5
# speedup vs baseline: 1.8761x; 1.0849x over previous
"""Trainium2 Bass kernel: per-gaussian 3x3 covariance from quaternion+scale.

out_n = R_n diag((|s_n|+eps)^2) R_n^T  with R_n from normalized quaternion.

Math: with raw (unnormalized) quaternion q=(w,x,y,z), n2=|q|^2, the matrix
M = n2*R has polynomial entries (no normalization needed):
  M00 = n2-(2y^2+2z^2)   M01 = 2xy-2wz   M02 = 2xz+2wy
  M10 = 2xy+2wz          M11 = n2-(2x^2+2z^2)   M12 = 2yz-2wx
  M20 = 2xz-2wy          M21 = 2yz+2wx   M22 = n2-(2x^2+2y^2)
With u_j = s_j/n2, B = M diag(u) = R diag(s_j), so out = B B^T
(column signs cancel in B B^T, and eps=1e-8 is numerically negligible).

Layout: host transposes inputs to component-planar planes per core, device
computes 6 unique output planes (symmetric), host reassembles [N,3,3].
"""

import numpy as np

N_TOTAL = 4_000_000
N_CORES = 8
NC_RAW = N_TOTAL // N_CORES  # 500_000
P = 128
F = -(-NC_RAW // P)          # 3907 elements per partition
NC_PAD = P * F               # 500_096
W = 512                      # tile width along free dim

_COMPILED = None


def _build(repeat=1):
    import contextlib
    import concourse.bacc as bacc
    import concourse.mybir as mybir
    from concourse import tile

    fp32 = mybir.dt.float32
    Alu = mybir.AluOpType
    Act = mybir.ActivationFunctionType
    SQ2 = float(np.sqrt(2.0))

    nc = bacc.Bacc("TRN2", target_bir_lowering=False, debug=False,
                   num_devices=N_CORES)
    qt = nc.dram_tensor("qt", [4, NC_PAD], fp32, kind="ExternalInput")
    st = nc.dram_tensor("st", [3, NC_PAD], fp32, kind="ExternalInput")
    ot = nc.dram_tensor("ot", [6, NC_PAD], fp32, kind="ExternalOutput")

    qv = qt.ap().rearrange("c (p f) -> c p f", p=P)
    sv = st.ap().rearrange("c (p f) -> c p f", p=P)
    ov = ot.ap().rearrange("c (p f) -> c p f", p=P)

    with tile.TileContext(nc) as tc:
        loop_ctx = tc.For_i(0, repeat, 1) if repeat > 1 else contextlib.nullcontext()
        with loop_ctx, tc.tile_pool(name="pool", bufs=1) as pool:
            V = nc.vector
            A = nc.scalar

            def new(tag, w, bufs=2):
                return pool.tile([P, w], fp32, tag=tag, name=tag, bufs=bufs)

            off = 0
            while off < F:
                w = min(W, F - off)
                sl = slice(off, off + w)

                # ---- loads (component planes) ----
                tq = [new(f"q{i}", w, bufs=2) for i in range(4)]  # w,x,y,z
                for i in range(4):
                    nc.sync.dma_start(out=tq[i][:], in_=qv[i, :, sl])
                ts = [new(f"s{i}", w, bufs=2) for i in range(3)]
                for i in range(3):
                    nc.sync.dma_start(out=ts[i][:], in_=sv[i, :, sl])
                tw, tx, ty, tz = tq

                # ---- squares on ACT: sw=w^2, sx2=2x^2, sy2=2y^2, sz2=2z^2
                sw = new("sw", w)
                A.activation(sw[:], tw[:], Act.Square)
                sx2 = new("sx2", w)
                A.activation(sx2[:], tx[:], Act.Square, scale=SQ2)
                sy2 = new("sy2", w)
                A.activation(sy2[:], ty[:], Act.Square, scale=SQ2)
                sz2 = new("sz2", w)
                A.activation(sz2[:], tz[:], Act.Square, scale=SQ2)

                # ---- doubled cross products via fused (a*2)*b
                xy2 = new("xy2", w)
                V.scalar_tensor_tensor(xy2[:], tx[:], 2.0, ty[:], Alu.mult, Alu.mult)
                xz2 = new("xz2", w)
                V.scalar_tensor_tensor(xz2[:], tx[:], 2.0, tz[:], Alu.mult, Alu.mult)
                yz2 = new("yz2", w)
                V.scalar_tensor_tensor(yz2[:], ty[:], 2.0, tz[:], Alu.mult, Alu.mult)
                wx2 = new("wx2", w)
                V.scalar_tensor_tensor(wx2[:], tw[:], 2.0, tx[:], Alu.mult, Alu.mult)
                wy2 = new("wy2", w)
                V.scalar_tensor_tensor(wy2[:], tw[:], 2.0, ty[:], Alu.mult, Alu.mult)
                wz2 = new("wz2", w)
                V.scalar_tensor_tensor(wz2[:], tw[:], 2.0, tz[:], Alu.mult, Alu.mult)

                # ---- diagonal helpers
                # t1 = 2(x²+y²+z²); g = w² − t1/2; n2 = g + t1 = |q|²
                # M_ii = n2 − (t1 − s_i2) = g + s_i2
                e2 = new("e2", w)
                V.tensor_tensor(e2[:], sx2[:], sy2[:], Alu.add)
                t1 = new("t1", w)
                V.tensor_tensor(t1[:], e2[:], sz2[:], Alu.add)
                g = new("g", w)
                V.scalar_tensor_tensor(g[:], t1[:], -0.5, sw[:], Alu.mult, Alu.add)
                n2 = new("n2", w)
                V.tensor_tensor(n2[:], g[:], t1[:], Alu.add)

                inv = new("inv", w)
                V.reciprocal_approx_fast(out=inv[:], in_=n2[:])

                u = [new(f"u{j}", w) for j in range(3)]
                for j in range(3):
                    V.tensor_tensor(u[j][:], ts[j][:], inv[:], Alu.mult)

                # ---- M entries
                m = {}
                for (key, aa, bb, op) in (
                    ("00", g, sx2, Alu.add),
                    ("11", g, sy2, Alu.add),
                    ("22", g, sz2, Alu.add),
                    ("01", xy2, wz2, Alu.subtract),
                    ("10", xy2, wz2, Alu.add),
                    ("02", xz2, wy2, Alu.add),
                    ("20", xz2, wy2, Alu.subtract),
                    ("12", yz2, wx2, Alu.subtract),
                    ("21", yz2, wx2, Alu.add),
                ):
                    t = new(f"m{key}", w)
                    V.tensor_tensor(t[:], aa[:], bb[:], op)
                    m[key] = t

                # ---- B = M diag(u)  (B == R diag(s))
                b = {}
                for i in range(3):
                    for j in range(3):
                        t = new(f"b{i}{j}", w)
                        V.tensor_tensor(t[:], m[f"{i}{j}"][:], u[j][:], Alu.mult)
                        b[(i, j)] = t

                # ---- diagonal outputs via ACT squares
                couts = {}
                dtags = ["xy2", "xz2", "yz2", "wx2", "wy2", "wz2", "e2", "t1", "g"]
                for i in range(3):
                    d0 = new(dtags[3 * i + 0], w)
                    A.activation(d0[:], b[(i, 0)][:], Act.Square)
                    d1 = new(dtags[3 * i + 1], w)
                    A.activation(d1[:], b[(i, 1)][:], Act.Square)
                    d2 = new(dtags[3 * i + 2], w)
                    A.activation(d2[:], b[(i, 2)][:], Act.Square)
                    ca = new(f"q{i}", w)
                    V.tensor_tensor(ca[:], d0[:], d1[:], Alu.add)
                    cd = new(f"c{i}{i}", w, bufs=2)
                    V.tensor_tensor(cd[:], ca[:], d2[:], Alu.add)
                    couts[(i, i)] = cd

                # ---- off-diagonal outputs
                ptags = {(0, 1): ("sw", "sx2", "n2", "sy2"),
                         (0, 2): ("sz2", "inv", "u0", "u1"),
                         (1, 2): ("u2", "q3", "s0", "s1")}
                for (i, k) in ((0, 1), (0, 2), (1, 2)):
                    tg = ptags[(i, k)]
                    p0 = new(tg[0], w)
                    V.tensor_tensor(p0[:], b[(i, 0)][:], b[(k, 0)][:], Alu.mult)
                    p1 = new(tg[1], w)
                    V.tensor_tensor(p1[:], b[(i, 1)][:], b[(k, 1)][:], Alu.mult)
                    p01 = new(tg[2], w)
                    V.tensor_tensor(p01[:], p0[:], p1[:], Alu.add)
                    p2 = new(tg[3], w)
                    V.tensor_tensor(p2[:], b[(i, 2)][:], b[(k, 2)][:], Alu.mult)
                    co = new(f"c{i}{k}", w, bufs=2)
                    V.tensor_tensor(co[:], p01[:], p2[:], Alu.add)
                    couts[(i, k)] = co

                # ---- stores: plane order c00,c01,c02,c11,c12,c22
                for plane, key in enumerate(((0, 0), (0, 1), (0, 2),
                                             (1, 1), (1, 2), (2, 2))):
                    nc.sync.dma_start(out=ov[plane, :, sl], in_=couts[key][:])

                off += w

    nc.compile()
    return nc


def _get_compiled():
    global _COMPILED
    if _COMPILED is None:
        _COMPILED = _build()
    return _COMPILED


def kernel(quaternion, scale):
    from concourse.bass_utils import run_bass_kernel_spmd

    q = np.ascontiguousarray(np.asarray(quaternion, dtype=np.float32))
    s = np.ascontiguousarray(np.asarray(scale, dtype=np.float32))
    assert q.shape == (N_TOTAL, 4) and s.shape == (N_TOTAL, 3)

    in_maps = []
    for c in range(N_CORES):
        lo, hi = c * NC_RAW, (c + 1) * NC_RAW
        qt = np.empty((4, NC_PAD), np.float32)
        qt[:, :NC_RAW] = q[lo:hi].T
        qt[0, NC_RAW:] = 1.0  # pad with identity quaternion
        qt[1:, NC_RAW:] = 0.0
        stt = np.ones((3, NC_PAD), np.float32)
        stt[:, :NC_RAW] = s[lo:hi].T
        in_maps.append({"qt": qt, "st": stt})

    nc = _get_compiled()
    res = run_bass_kernel_spmd(nc, in_maps, core_ids=list(range(N_CORES)))

    out = np.empty((N_TOTAL, 3, 3), np.float32)
    # plane -> (i,j) positions (symmetric duplicates share a plane)
    placement = ((0, (0, 0)), (1, (0, 1)), (2, (0, 2)),
                 (1, (1, 0)), (3, (1, 1)), (4, (1, 2)),
                 (2, (2, 0)), (4, (2, 1)), (5, (2, 2)))
    for c in range(N_CORES):
        o6 = res.results[c]["ot"]
        lo = c * NC_RAW
        for plane, (i, j) in placement:
            out[lo:lo + NC_RAW, i, j] = o6[plane, :NC_RAW]
    return out
